# revision 20
# baseline (speedup 1.0000x reference)
# Trainium2 Bass kernel for the FFM (fast-forgetful-memory) layer.
#
# Math: the reference does a complex segmented scan  s[t] = z[t] + !r[t] * gamma * s[t-1]
# over channels (trace=64) x (ctx=64), gamma = exp(-|a_tr| + i*b_c), followed by a
# big mixing matmul.  We substitute u[t] = exp(-i*b_c*t) * s[t], which turns the
# complex recurrence into two independent REAL first-order scans
#     u[t] = exp(-|a|) * !r[t] * u[t-1] + exp(-i*b*t) * g[t]
# (numerically stable for the whole sequence since |exp(-i*b*t)| = 1).  These map
# directly onto the DVE tensor_tensor_scan instruction.
#
# The output rotation s = exp(i*b*t) * u is folded into the mixing matmul: with
# sn := -sin(b*t),
#   zm = sum_ch s_re*Wr + s_im*Wi
#      = sum_ch (cos.u_re)Wr + (cos.u_im)Wi + (sn.u_re)(-Wi) + (sn.u_im)Wr
# so only TWO elementwise products per u component are needed (cos.u and sn.u),
# each contracted against a normal (A) and a swapped/negated (B) copy of W_mix.
#
# Sharding: time is split across the 8 cores (512 steps each).  Each core runs
# zero-initialized local scans, per-core carry summaries are exchanged with two
# batched AllGathers (latency hidden behind the remaining scans), a short
# on-device chain reconstructs each core's true initial state, and a rank-1
# correction (on GPSIMD) fixes the local scans.
import numpy as np

import concourse.bass as bass
import concourse.bacc as bacc
import concourse.mybir as mybir
import concourse.tile as tile
from concourse.bass_utils import run_bass_kernel_spmd

T, D, TR, CX, OUT = 4096, 512, 64, 64, 512
NCORE, TL, G = 8, 512, 32
LN_EPS = 1e-6
W4 = 4 * TL   # 4-group slab width (2048)

F32 = mybir.dt.float32
BF16 = mybir.dt.bfloat16
NP_BF16 = mybir.dt.np(BF16)
MULT = mybir.AluOpType.mult
ADD = mybir.AluOpType.add
SUB = mybir.AluOpType.subtract
AF = mybir.ActivationFunctionType

_P = np.arange(128)
TR_OF_P = _P % 64            # trace index per partition
HI_OF_P = _P // 64           # 0/1 selecting c = 2j + HI


def _build_program():
    nc = bacc.Bacc("TRN2", target_bir_lowering=False, debug=False, num_devices=NCORE)

    def inp(name, shape, dtype):
        return nc.dram_tensor(name, shape, dtype, kind="ExternalInput").ap()

    xT = inp("xT", [D, TL], BF16)
    wpre2 = inp("wpre2", [D, 128], BF16)
    wgi2 = inp("wgi2", [D, 128], BF16)
    bprecol = inp("bprecol", [128, 1], F32)
    bgicol = inp("bgicol", [128, 1], F32)
    wgo = inp("wgo", [D, OUT], BF16)
    wskip = inp("wskip", [D, OUT], BF16)
    brow = inp("brow", [3, OUT], F32)
    wmix = inp("wmix", [8, 128, 16 * OUT], BF16)  # consumption-ordered mega-blocks
    cosb = inp("cosb", [128, G * TL], BF16)
    sinb = inp("sinb", [128, G * TL], BF16)     # = -sin(b*t)
    dect = inp("dect", [128, TL], BF16)
    mrow = inp("mrow", [1, W4], F32)            # notreset, zeroed at 512-boundaries
    dcol = inp("dcol", [128, 1], F32)
    init0 = inp("init0", [128, 2 * G], F32)
    mf2 = inp("mf2", [128, NCORE], F32)
    mlf = inp("mlf", [128, NCORE], F32)

    out_d = nc.dram_tensor("out", [TL, OUT], F32, kind="ExternalOutput").ap()
    lfout = nc.dram_tensor("lfout", [128, 2 * G], F32, kind="ExternalOutput").ap()

    with tile.TileContext(nc) as tc:
        with (
            tc.tile_pool(name="big", bufs=1) as big,
            tc.tile_pool(name="const", bufs=1) as const,
            tc.tile_pool(name="slab", bufs=1) as slab,
            tc.tile_pool(name="wmixp", bufs=2) as wmixp,
            tc.tile_pool(name="gp", bufs=3) as gp,
            tc.tile_pool(name="smallp", bufs=2) as smallp,
            tc.tile_pool(name="pz", bufs=1, space="PSUM") as pz,
            tc.tile_pool(name="pg", bufs=1, space="PSUM") as pg,
            tc.tile_pool(name="dramp", bufs=1, space="DRAM") as dramp,
        ):
            # ---------------- persistent loads ----------------
            uRe = [big.tile([128, 8 * TL], BF16, tag=f"uRe{s}", name=f"uRe{s}") for s in range(4)]
            uIm = [big.tile([128, 8 * TL], BF16, tag=f"uIm{s}", name=f"uIm{s}") for s in range(4)]

            wgoT, wskT = [], []
            for kc in range(4):
                t = const.tile([128, OUT], BF16, tag=f"wgo{kc}", name=f"wgoT{kc}")
                nc.sync.dma_start(out=t[:], in_=wgo[kc * 128:(kc + 1) * 128, :])
                wgoT.append(t)
                t = const.tile([128, OUT], BF16, tag=f"wsk{kc}", name=f"wskT{kc}")
                nc.sync.dma_start(out=t[:], in_=wskip[kc * 128:(kc + 1) * 128, :])
                wskT.append(t)
            dectT = const.tile([128, TL], BF16, tag="dect")
            nc.sync.dma_start(out=dectT[:], in_=dect[:])
            mrowT = const.tile([1, W4], F32, tag="mrow")
            nc.sync.dma_start(out=mrowT[:], in_=mrow[:])
            dcolT = const.tile([128, 1], F32, tag="dcol")
            nc.sync.dma_start(out=dcolT[:], in_=dcol[:])
            bpreT = const.tile([128, 1], F32, tag="bpre")
            nc.sync.dma_start(out=bpreT[:], in_=bprecol[:])
            bgiT = const.tile([128, 1], F32, tag="bgi")
            nc.sync.dma_start(out=bgiT[:], in_=bgicol[:])
            bgoR = const.tile([1, OUT], F32, tag="bgoR")
            nc.sync.dma_start(out=bgoR[:], in_=brow[0:1, :])
            bskR = const.tile([1, OUT], F32, tag="bskR")
            nc.sync.dma_start(out=bskR[:], in_=brow[1:2, :])
            bmixR = const.tile([1, OUT], F32, tag="bmixR")
            nc.sync.dma_start(out=bmixR[:], in_=brow[2:3, :])
            bgoRep = const.tile([128, OUT], F32, tag="bgoRep")
            nc.gpsimd.partition_broadcast(bgoRep[:], bgoR[:])
            bskRep = const.tile([128, OUT], F32, tag="bskRep")
            nc.gpsimd.partition_broadcast(bskRep[:], bskR[:])
            bmixRep = const.tile([128, OUT], F32, tag="bmixRep")
            nc.gpsimd.partition_broadcast(bmixRep[:], bmixR[:])
            init0T = const.tile([128, 2 * G], F32, tag="init0")
            nc.sync.dma_start(out=init0T[:], in_=init0[:])
            mf2T = const.tile([128, NCORE], F32, tag="mf2")
            nc.sync.dma_start(out=mf2T[:], in_=mf2[:])
            mlfT = const.tile([128, NCORE], F32, tag="mlf")
            nc.sync.dma_start(out=mlfT[:], in_=mlf[:])

            # ---------------- g = pre * sigmoid(gi), doubled over partitions ----
            ppre = pg.tile([128, TL], F32, tag="ppre")
            pgi = pg.tile([128, TL], F32, tag="pgi")
            for kc in range(4):
                xc = slab.tile([128, TL], BF16, tag="xc", bufs=2, name=f"xc{kc}")
                nc.sync.dma_start(out=xc[:], in_=xT[kc * 128:(kc + 1) * 128, :])
                wt = const.tile([128, 128], BF16, tag=f"wpre{kc}", name=f"wt_{kc}")
                nc.sync.dma_start(out=wt[:], in_=wpre2[kc * 128:(kc + 1) * 128, :])
                nc.tensor.matmul(ppre[:], wt[:], xc[:], start=(kc == 0), stop=(kc == 3))
                wt2 = const.tile([128, 128], BF16, tag=f"wgi{kc}", name=f"wt2_{kc}")
                nc.sync.dma_start(out=wt2[:], in_=wgi2[kc * 128:(kc + 1) * 128, :])
                nc.tensor.matmul(pgi[:], wt2[:], xc[:], start=(kc == 0), stop=(kc == 3))
            preB = gp.tile([128, TL], F32, tag="workf", name="preB")
            nc.scalar.activation(preB[:], ppre[:], AF.Identity, bias=bpreT[:], scale=1.0)
            giS = gp.tile([128, TL], F32, tag="workf", name="giS")
            nc.scalar.activation(giS[:], pgi[:], AF.Sigmoid, bias=bgiT[:], scale=1.0)
            gRep = const.tile([128, TL], BF16, tag="gRep")
            nc.vector.tensor_tensor(out=gRep[:], in0=preB[:], in1=giS[:], op=MULT)
            gRep4 = gRep[:].rearrange("p (o t) -> p o t", o=1).broadcast_to([128, 4, TL])

            # ---------------- m slab (4 groups wide, boundary-zeroed) ----------
            mBig = const.tile([128, W4], F32, tag="mBig")
            nc.gpsimd.partition_broadcast(mBig[:], mrowT[:])
            nc.vector.tensor_scalar_mul(mBig[:], mBig[:], dcolT[:])

            # ---------------- per-slab pipeline: w/scan -> AG -> chain -> corr --
            # lf/init col layout per slab: col 2*jl (+1 for imag), group j = 8*s + jl
            curs = []
            for v in range(8):
                s, h = v // 2, v % 2
                hsl = slice(h * W4, (h + 1) * W4)
                csl = slice(v * W4, (v + 1) * W4)
                cosS = slab.tile([128, W4], BF16, tag="tb", bufs=4, name=f"cosSw{v}")
                nc.sync.dma_start(out=cosS[:], in_=cosb[:, csl])
                sinS = slab.tile([128, W4], BF16, tag="tb", bufs=4, name=f"sinSw{v}")
                nc.sync.dma_start(out=sinS[:], in_=sinb[:, csl])
                wre = slab.tile([128, W4], BF16, tag="ws", bufs=3, name="wre")
                wim = slab.tile([128, W4], BF16, tag="ws", bufs=3, name="wim")
                weng = nc.vector if v % 2 == 0 else nc.gpsimd
                weng.tensor_tensor(
                    out=wre[:].rearrange("p (o t) -> p o t", o=4),
                    in0=gRep4, in1=cosS[:].rearrange("p (o t) -> p o t", o=4),
                    op=MULT)
                weng.tensor_tensor(
                    out=wim[:].rearrange("p (o t) -> p o t", o=4),
                    in0=gRep4, in1=sinS[:].rearrange("p (o t) -> p o t", o=4),
                    op=MULT)
                nc.vector.tensor_tensor_scan(
                    out=uRe[s][:, hsl], data0=mBig[:], data1=wre[:],
                    initial=0.0, op0=MULT, op1=ADD)
                nc.vector.tensor_tensor_scan(
                    out=uIm[s][:, hsl], data0=mBig[:], data1=wim[:],
                    initial=0.0, op0=MULT, op1=ADD)

                if v % 2 == 0:
                    continue
                # slab s complete: local finals -> AllGather -> chain -> correction
                lfS = const.tile([128, 16], F32, tag=f"lfS{s}", name=f"lfS{s}")
                nc.scalar.copy(out=lfS[:, 0:16:2], in_=uRe[s][:, TL - 1:: TL])
                nc.scalar.copy(out=lfS[:, 1:16:2], in_=uIm[s][:, TL - 1:: TL])
                nc.sync.dma_start(out=lfout[:, 16 * s:16 * (s + 1)], in_=lfS[:])
                lf_in_t = dramp.tile([128, 16], F32, tag=f"lf_in{s}", name=f"lf_in{s}")
                lf_all_t = dramp.tile([NCORE * 128, 16], F32, tag=f"lf_all{s}",
                                      name=f"lf_all{s}")
                nc.sync.dma_start(out=lf_in_t[:], in_=lfS[:])
                nc.gpsimd.collective_compute(
                    "AllGather", mybir.AluOpType.bypass,
                    replica_groups=[list(range(NCORE))],
                    ins=[lf_in_t.opt()], outs=[lf_all_t.opt()])
                lfAll = const.tile([128, NCORE * 16], F32, tag=f"lfAll{s}",
                                   name=f"lfAll{s}")
                nc.sync.dma_start(
                    out=lfAll[:].rearrange("p (r c) -> p r c", r=NCORE),
                    in_=lf_all_t[:].rearrange("(r p) c -> p r c", p=128))
                # chain: steps >= own core index are identity (data-driven)
                cur = init0T[:, 16 * s:16 * (s + 1)]
                for bb in range(NCORE):
                    tmp = smallp.tile([128, 16], F32, tag="chA", name=f"chA{s}_{bb}")
                    nc.vector.tensor_scalar_mul(tmp[:], cur, mf2T[:, bb:bb + 1])
                    nxt = smallp.tile([128, 16], F32, tag="chB", name=f"chB{s}_{bb}")
                    nc.vector.scalar_tensor_tensor(
                        out=nxt[:], in0=lfAll[:, bb * 16:(bb + 1) * 16],
                        scalar=mlfT[:, bb:bb + 1], in1=tmp[:], op0=MULT, op1=ADD)
                    cur = nxt[:]
                curB = smallp.tile([128, 16], BF16, tag="curB", name=f"curB{s}")
                nc.vector.tensor_copy(curB[:], cur)
                curs.append(curB[:])
                for jl in range(8):
                    jsl = slice(jl * TL, (jl + 1) * TL)
                    nc.vector.scalar_tensor_tensor(
                        out=uRe[s][:, jsl], in0=dectT[:], scalar=curB[:, 2 * jl:2 * jl + 1],
                        in1=uRe[s][:, jsl], op0=MULT, op1=ADD)
                    nc.vector.scalar_tensor_tensor(
                        out=uIm[s][:, jsl], in0=dectT[:], scalar=curB[:, 2 * jl + 1:2 * jl + 2],
                        in1=uIm[s][:, jsl], op0=MULT, op1=ADD)

            # ---------------- rotate-products + mixing matmul ----------------
            # zm = (cos.u)@A + (sn.u)@B with A/B chunk pairs per group
            zmP = [pz.tile([128, OUT], F32, tag=f"zm{t}", name=f"zmP{t}") for t in range(4)]
            for v in range(8):
                s, h = v // 2, v % 2
                hsl = slice(h * W4, (h + 1) * W4)
                csl = slice(v * W4, (v + 1) * W4)
                cosS = slab.tile([128, W4], BF16, tag="tb", bufs=4, name=f"cosSr{v}")
                nc.sync.dma_start(out=cosS[:], in_=cosb[:, csl])
                sinS = slab.tile([128, W4], BF16, tag="tb", bufs=4, name=f"sinSr{v}")
                nc.sync.dma_start(out=sinS[:], in_=sinb[:, csl])
                prods = []
                for name, uu, tabS in (("qcre", uRe[s], cosS), ("qcim", uIm[s], cosS),
                                       ("qsre", uRe[s], sinS), ("qsim", uIm[s], sinS)):
                    q = slab.tile([128, W4], BF16, tag="qs", bufs=5, name=f"{name}{v}")
                    nc.vector.tensor_tensor(out=q[:], in0=tabS[:],
                                            in1=uu[:, hsl], op=MULT)
                    prods.append(q)
                wmt = wmixp.tile([128, 16 * OUT], BF16, tag="wmt", name=f"wmt{v}")
                nc.sync.dma_start(out=wmt[:], in_=wmix[v])
                for jj in range(4):
                    j = 4 * v + jj
                    # block order: (qc_re, qc_im, qs_re, qs_im) per group
                    for qi, q in enumerate(prods):
                        idx = 4 * jj + qi
                        wsl = slice(idx * OUT, (idx + 1) * OUT)
                        first = (v == 0 and jj == 0 and qi == 0)
                        for tau in range(4):
                            nc.tensor.matmul(
                                zmP[tau][:],
                                q[:, jj * TL + tau * 128: jj * TL + (tau + 1) * 128],
                                wmt[:, wsl],
                                start=first, stop=(v == 7 and jj == 3 and qi == 3))
            # ---------------- gates / layernorm tail ----------------
            for tau in range(4):
                tsl = slice(tau * 128, (tau + 1) * 128)
                goP = pg.tile([128, OUT], F32, tag="goP")
                skP = pg.tile([128, OUT], F32, tag="skP")
                xg = []
                for kc in range(4):
                    xgt = slab.tile([128, 128], BF16, tag="xg", bufs=8, name=f"xg{tau}_{kc}")
                    nc.sync.dma_start(out=xgt[:], in_=xT[kc * 128:(kc + 1) * 128, tsl])
                    xg.append(xgt)
                    nc.tensor.matmul(goP[:], xgt[:], wgoT[kc][:],
                                     start=(kc == 0), stop=(kc == 3))
                for kc in range(4):
                    nc.tensor.matmul(skP[:], xg[kc][:], wskT[kc][:],
                                     start=(kc == 0), stop=(kc == 3))

                go2 = gp.tile([128, OUT], F32, tag="workf", name="go2")
                nc.vector.tensor_tensor(out=go2[:], in0=goP[:], in1=bgoRep[:], op=ADD)
                gate = gp.tile([128, OUT], F32, tag="gate", bufs=1)
                nc.scalar.activation(gate[:], go2[:], AF.Sigmoid)
                skipS = gp.tile([128, OUT], F32, tag="skipS", bufs=1)
                nc.vector.tensor_tensor(out=skipS[:], in0=skP[:], in1=bskRep[:], op=ADD)
                z2 = gp.tile([128, OUT], F32, tag="workf", name="z2")
                nc.vector.tensor_tensor(out=z2[:], in0=zmP[tau][:], in1=bmixRep[:], op=ADD)
                v_ = gp.tile([128, OUT], F32, tag="workf", name="v_")
                nc.vector.tensor_tensor(out=v_[:], in0=z2[:], in1=gate[:], op=MULT)
                musum = smallp.tile([128, 1], F32, tag="musum")
                nc.vector.tensor_reduce(out=musum[:], in_=v_[:], axis=mybir.AxisListType.X, op=ADD)
                negmu = smallp.tile([128, 1], F32, tag="negmu")
                nc.vector.tensor_scalar_mul(negmu[:], musum[:], -1.0 / OUT)
                cen = gp.tile([128, OUT], F32, tag="workf", name="cen")
                nc.scalar.activation(cen[:], v_[:], AF.Identity, bias=negmu[:], scale=1.0)
                sqj = gp.tile([128, OUT], BF16, tag="sqj", bufs=1)
                varsum = smallp.tile([128, 1], F32, tag="varsum")
                nc.scalar.activation(sqj[:], cen[:], AF.Square, accum_out=varsum[:])
                varm = smallp.tile([128, 1], F32, tag="varm")
                nc.vector.tensor_scalar(out=varm[:], in0=varsum[:], scalar1=1.0 / OUT,
                                        scalar2=LN_EPS, op0=MULT, op1=ADD)
                stdc = smallp.tile([128, 1], F32, tag="stdc")
                nc.scalar.activation(stdc[:], varm[:], AF.Sqrt)
                rstd = smallp.tile([128, 1], F32, tag="rstd")
                nc.vector.reciprocal(rstd[:], stdc[:])
                ln = gp.tile([128, OUT], F32, tag="workf", name="ln")
                nc.vector.tensor_scalar_mul(ln[:], cen[:], rstd[:])
                omg = gp.tile([128, OUT], F32, tag="workf", name="omg")
                nc.vector.tensor_scalar(out=omg[:], in0=gate[:], scalar1=-1.0,
                                        scalar2=1.0, op0=MULT, op1=ADD)
                t5 = gp.tile([128, OUT], F32, tag="workf", name="t5")
                nc.vector.tensor_tensor(out=t5[:], in0=skipS[:], in1=omg[:], op=MULT)
                outT = gp.tile([128, OUT], F32, tag="workf", name="outT")
                nc.vector.tensor_tensor(out=outT[:], in0=ln[:], in1=t5[:], op=ADD)
                nc.sync.dma_start(out=out_d[tsl, :], in_=outT[:])

    nc.finalize()
    return nc


_NC_CACHE = {}


def _get_nc():
    if "nc" not in _NC_CACHE:
        _NC_CACHE["nc"] = _build_program()
    return _NC_CACHE["nc"]


def _host_prep(inputs):
    f8 = np.float64
    x = np.asarray(inputs["x"], f8)
    resets = np.asarray(inputs["resets"]).astype(bool)
    a = np.abs(np.asarray(inputs["a"], f8))
    b = np.asarray(inputs["b"], f8)
    s0 = (np.asarray(inputs["state_re"], f8)[0]
          + 1j * np.asarray(inputs["state_im"], f8)[0])      # [TR, CX]

    W_pre = np.asarray(inputs["W_pre"], f8)
    W_gi = np.asarray(inputs["W_gi"], f8)
    wpre2 = np.concatenate([W_pre, W_pre], 1).astype(NP_BF16)
    wgi2 = np.concatenate([W_gi, W_gi], 1).astype(NP_BF16)
    bprecol = np.tile(np.asarray(inputs["b_pre"], f8), 2)[:, None].astype(np.float32)
    bgicol = np.tile(np.asarray(inputs["b_gi"], f8), 2)[:, None].astype(np.float32)
    wgo = np.asarray(inputs["W_go"], f8).astype(NP_BF16)
    wskip = np.asarray(inputs["W_skip"], f8).astype(NP_BF16)
    brow = np.stack([np.asarray(inputs["b_go"], f8),
                     np.asarray(inputs["b_skip"], f8),
                     np.asarray(inputs["b_mix"], f8)]).astype(np.float32)

    # channel permutation tables
    c_of = 2 * np.arange(G)[None, :] + HI_OF_P[:, None]       # [128, G]
    bmat = b[c_of]                                            # [128, G]
    Wm = np.asarray(inputs["W_mix"], f8)
    # mega-blocks: block v holds, for j = 4v..4v+3, the four OUT-wide chunks
    # pairing (cos.u_re, cos.u_im, sn.u_re, sn.u_im) = (Wr, Wi, -Wi, Wr)
    wmix = np.empty((8, 128, 16 * OUT), NP_BF16)
    for j in range(G):
        c = c_of[:, j]
        Wr = Wm[TR_OF_P * 128 + c]
        Wi = Wm[TR_OF_P * 128 + 64 + c]
        v, jj = j // 4, j % 4
        wmix[v, :, (4 * jj + 0) * OUT:(4 * jj + 1) * OUT] = Wr.astype(NP_BF16)
        wmix[v, :, (4 * jj + 1) * OUT:(4 * jj + 2) * OUT] = Wi.astype(NP_BF16)
        wmix[v, :, (4 * jj + 2) * OUT:(4 * jj + 3) * OUT] = (-Wi).astype(NP_BF16)
        wmix[v, :, (4 * jj + 3) * OUT:(4 * jj + 4) * OUT] = Wr.astype(NP_BF16)

    decay = np.exp(-a)                                        # [TR]
    dcol = decay[TR_OF_P][:, None].astype(np.float32)
    anyr = np.array([resets[k * TL:(k + 1) * TL].any() for k in range(NCORE)])
    Mf_part = np.exp(-TL * a[TR_OF_P])                        # [128]

    init0c = np.exp(1j * bmat) * s0[TR_OF_P[:, None], c_of]   # [128, G] complex
    # lf/init column layout: col 16*s + 2*jj (+1) for group j = 8*s + jj
    col_of_j = np.array([16 * (j // 8) + 2 * (j % 8) for j in range(G)], np.int64)
    init0 = np.empty((128, 2 * G), np.float32)
    init0[:, col_of_j] = init0c.real
    init0[:, col_of_j + 1] = init0c.imag

    in_maps = []
    for k in range(NCORE):
        tg = np.arange(k * TL, (k + 1) * TL, dtype=f8)        # global t
        ph = bmat[:, :, None] * tg[None, None, :]             # [128, G, TL]
        cosb = np.cos(ph).reshape(128, G * TL).astype(NP_BF16)
        sinb = (-np.sin(ph)).reshape(128, G * TL).astype(NP_BF16)
        notr = (~resets[k * TL:(k + 1) * TL]).astype(f8)      # [TL]
        cumnr = np.cumprod(notr)
        dect = (np.exp(-a[TR_OF_P][:, None] * (np.arange(TL)[None, :] + 1))
                * cumnr[None, :]).astype(NP_BF16)
        mrow = np.tile(notr, 4).astype(np.float32)
        mrow[0::TL] = 0.0                                     # group boundaries
        mf2 = np.empty((128, NCORE), np.float32)
        mlf = np.empty((128, NCORE), np.float32)
        for bb in range(NCORE):
            if bb < k:
                mf2[:, bb] = Mf_part * (0.0 if anyr[bb] else 1.0)
                mlf[:, bb] = 1.0
            else:
                mf2[:, bb] = 1.0
                mlf[:, bb] = 0.0
        in_maps.append(dict(
            xT=np.ascontiguousarray(x[k * TL:(k + 1) * TL].T).astype(NP_BF16),
            wpre2=wpre2, wgi2=wgi2, bprecol=bprecol, bgicol=bgicol,
            wgo=wgo, wskip=wskip, brow=brow, wmix=wmix,
            cosb=cosb, sinb=sinb, dect=dect,
            mrow=mrow[None, :],
            dcol=dcol, init0=init0, mf2=mf2, mlf=mlf,
        ))
    aux = dict(bmat=bmat, Mf_part=Mf_part, anyr=anyr, init0c=init0c, c_of=c_of,
               col_of_j=col_of_j)
    return in_maps, aux


def _assemble(results, aux):
    out = np.concatenate([results[k]["out"] for k in range(NCORE)], 0).astype(np.float32)

    # final state: chain the device-produced local finals on the host
    col = aux["col_of_j"]
    init = aux["init0c"].astype(np.complex128)                # [128, G]
    for k in range(NCORE):
        lf = results[k]["lfout"]
        lfc = lf[:, col] + 1j * lf[:, col + 1]
        init = lfc + (0.0 if aux["anyr"][k] else 1.0) * aux["Mf_part"][:, None] * init
    sfin = init * np.exp(1j * aux["bmat"] * (T - 1))
    fin = np.zeros((TR, CX), np.complex64)
    fin[TR_OF_P[:, None], aux["c_of"]] = sfin.astype(np.complex64)
    return fin[None], out


def kernel(**inputs):
    nc = _get_nc()
    in_maps, aux = _host_prep(inputs)
    res = run_bass_kernel_spmd(nc, in_maps, list(range(NCORE)))
    return _assemble(res.results, aux)


# revision 23
# speedup vs baseline: 1.2510x; 1.2510x over previous
# Trainium2 Bass kernel for the FFM (fast-forgetful-memory) layer.
#
# Math: the reference does a complex segmented scan  s[t] = z[t] + !r[t] * gamma * s[t-1]
# over channels (trace=64) x (ctx=64), gamma = exp(-|a_tr| + i*b_c), followed by a
# big mixing matmul.  We substitute u[t] = exp(-i*b_c*t) * s[t], which turns the
# complex recurrence into two independent REAL first-order scans
#     u[t] = exp(-|a|) * !r[t] * u[t-1] + exp(-i*b*t) * g[t]
# (numerically stable for the whole sequence since |exp(-i*b*t)| = 1).  These map
# directly onto the DVE tensor_tensor_scan instruction.
#
# The output rotation s = exp(i*b*t) * u is folded into the mixing matmul: with
# sn := -sin(b*t),
#   zm = sum_ch s_re*Wr + s_im*Wi
#      = sum_ch (cos.u_re)Wr + (cos.u_im)Wi + (sn.u_re)(-Wi) + (sn.u_im)Wr
# so only TWO elementwise products per u component are needed (cos.u and sn.u),
# each contracted against a normal (A) and a swapped/negated (B) copy of W_mix.
#
# Sharding: time is split across the 8 cores (512 steps each).  Each core runs
# zero-initialized local scans, per-core carry summaries are exchanged with two
# batched AllGathers (latency hidden behind the remaining scans), a short
# on-device chain reconstructs each core's true initial state, and a rank-1
# correction (on GPSIMD) fixes the local scans.
import numpy as np

import concourse.bass as bass
import concourse.bacc as bacc
import concourse.mybir as mybir
import concourse.tile as tile
from concourse.bass_utils import run_bass_kernel_spmd

T, D, TR, CX, OUT = 4096, 512, 64, 64, 512
NCORE, TL, G = 8, 512, 32
LN_EPS = 1e-6
W4 = 4 * TL   # 4-group slab width (2048)

F32 = mybir.dt.float32
BF16 = mybir.dt.bfloat16
NP_BF16 = mybir.dt.np(BF16)
MULT = mybir.AluOpType.mult
ADD = mybir.AluOpType.add
SUB = mybir.AluOpType.subtract
AF = mybir.ActivationFunctionType

_P = np.arange(128)
TR_OF_P = _P % 64            # trace index per partition
HI_OF_P = _P // 64           # 0/1 selecting c = 2j + HI


def _build_program():
    nc = bacc.Bacc("TRN2", target_bir_lowering=False, debug=False, num_devices=NCORE)

    def inp(name, shape, dtype):
        return nc.dram_tensor(name, shape, dtype, kind="ExternalInput").ap()

    xT = inp("xT", [D, TL], BF16)
    wpre2 = inp("wpre2", [D, 128], BF16)
    wgi2 = inp("wgi2", [D, 128], BF16)
    bprecol = inp("bprecol", [128, 1], F32)
    bgicol = inp("bgicol", [128, 1], F32)
    wgo = inp("wgo", [D, OUT], BF16)
    wskip = inp("wskip", [D, OUT], BF16)
    brow = inp("brow", [3, OUT], F32)
    wmix = inp("wmix", [8, 128, 16 * OUT], BF16)  # consumption-ordered mega-blocks
    cosb = inp("cosb", [128, G * TL], BF16)
    sinb = inp("sinb", [128, G * TL], BF16)     # = -sin(b*t)
    dect = inp("dect", [128, TL], BF16)
    mrow = inp("mrow", [1, W4], F32)            # notreset, zeroed at 512-boundaries
    dcol = inp("dcol", [128, 1], F32)
    init0 = inp("init0", [128, 2 * G], F32)
    mf2 = inp("mf2", [128, NCORE], F32)
    mlf = inp("mlf", [128, NCORE], F32)

    out_d = nc.dram_tensor("out", [TL, OUT], F32, kind="ExternalOutput").ap()
    lfout = nc.dram_tensor("lfout", [128, 2 * G], F32, kind="ExternalOutput").ap()

    with tile.TileContext(nc) as tc:
        with (
            tc.tile_pool(name="big", bufs=1) as big,
            tc.tile_pool(name="const", bufs=1) as const,
            tc.tile_pool(name="slab", bufs=1) as slab,  # per-tag bufs overrides
            tc.tile_pool(name="wmixp", bufs=2) as wmixp,
            tc.tile_pool(name="gp", bufs=3) as gp,
            tc.tile_pool(name="smallp", bufs=2) as smallp,
            tc.tile_pool(name="pz", bufs=1, space="PSUM") as pz,
            tc.tile_pool(name="pg", bufs=1, space="PSUM") as pg,
            tc.tile_pool(name="dramp", bufs=1, space="DRAM") as dramp,
        ):
            # ---------------- persistent loads ----------------
            uRe = [big.tile([128, 8 * TL], BF16, tag=f"uRe{s}", name=f"uRe{s}") for s in range(4)]
            uIm = [big.tile([128, 8 * TL], BF16, tag=f"uIm{s}", name=f"uIm{s}") for s in range(4)]

            wgoT, wskT = [], []
            for kc in range(4):
                t = const.tile([128, OUT], BF16, tag=f"wgo{kc}", name=f"wgoT{kc}")
                nc.sync.dma_start(out=t[:], in_=wgo[kc * 128:(kc + 1) * 128, :])
                wgoT.append(t)
                t = const.tile([128, OUT], BF16, tag=f"wsk{kc}", name=f"wskT{kc}")
                nc.sync.dma_start(out=t[:], in_=wskip[kc * 128:(kc + 1) * 128, :])
                wskT.append(t)
            dectT = const.tile([128, TL], BF16, tag="dect")
            nc.sync.dma_start(out=dectT[:], in_=dect[:])
            mrowT = const.tile([1, W4], F32, tag="mrow")
            nc.sync.dma_start(out=mrowT[:], in_=mrow[:])
            dcolT = const.tile([128, 1], F32, tag="dcol")
            nc.sync.dma_start(out=dcolT[:], in_=dcol[:])
            bpreT = const.tile([128, 1], F32, tag="bpre")
            nc.sync.dma_start(out=bpreT[:], in_=bprecol[:])
            bgiT = const.tile([128, 1], F32, tag="bgi")
            nc.sync.dma_start(out=bgiT[:], in_=bgicol[:])
            bgoR = const.tile([1, OUT], F32, tag="bgoR")
            nc.sync.dma_start(out=bgoR[:], in_=brow[0:1, :])
            bskR = const.tile([1, OUT], F32, tag="bskR")
            nc.sync.dma_start(out=bskR[:], in_=brow[1:2, :])
            bmixR = const.tile([1, OUT], F32, tag="bmixR")
            nc.sync.dma_start(out=bmixR[:], in_=brow[2:3, :])
            bgoRep = const.tile([128, OUT], F32, tag="bgoRep")
            nc.gpsimd.partition_broadcast(bgoRep[:], bgoR[:])
            bskRep = const.tile([128, OUT], F32, tag="bskRep")
            nc.gpsimd.partition_broadcast(bskRep[:], bskR[:])
            bmixRep = const.tile([128, OUT], F32, tag="bmixRep")
            nc.gpsimd.partition_broadcast(bmixRep[:], bmixR[:])
            init0T = const.tile([128, 2 * G], F32, tag="init0")
            nc.sync.dma_start(out=init0T[:], in_=init0[:])
            mf2T = const.tile([128, NCORE], F32, tag="mf2")
            nc.sync.dma_start(out=mf2T[:], in_=mf2[:])
            mlfT = const.tile([128, NCORE], F32, tag="mlf")
            nc.sync.dma_start(out=mlfT[:], in_=mlf[:])

            # ---------------- g = pre * sigmoid(gi), doubled over partitions ----
            ppre = pg.tile([128, TL], F32, tag="ppre")
            pgi = pg.tile([128, TL], F32, tag="pgi")
            for kc in range(4):
                xc = slab.tile([128, TL], BF16, tag="xc", bufs=2, name=f"xc{kc}")
                nc.sync.dma_start(out=xc[:], in_=xT[kc * 128:(kc + 1) * 128, :])
                wt = const.tile([128, 128], BF16, tag=f"wpre{kc}", name=f"wt_{kc}")
                nc.sync.dma_start(out=wt[:], in_=wpre2[kc * 128:(kc + 1) * 128, :])
                nc.tensor.matmul(ppre[:], wt[:], xc[:], start=(kc == 0), stop=(kc == 3))
                wt2 = const.tile([128, 128], BF16, tag=f"wgi{kc}", name=f"wt2_{kc}")
                nc.sync.dma_start(out=wt2[:], in_=wgi2[kc * 128:(kc + 1) * 128, :])
                nc.tensor.matmul(pgi[:], wt2[:], xc[:], start=(kc == 0), stop=(kc == 3))
            preB = gp.tile([128, TL], F32, tag="workf", name="preB")
            nc.scalar.activation(preB[:], ppre[:], AF.Identity, bias=bpreT[:], scale=1.0)
            giS = gp.tile([128, TL], F32, tag="workf", name="giS")
            nc.scalar.activation(giS[:], pgi[:], AF.Sigmoid, bias=bgiT[:], scale=1.0)
            gRep = const.tile([128, TL], BF16, tag="gRep")
            nc.vector.tensor_tensor(out=gRep[:], in0=preB[:], in1=giS[:], op=MULT)
            gRep4p = const.tile([128, W4], BF16, tag="gRep4p")
            for o in range(4):
                nc.vector.tensor_copy(gRep4p[:, o * TL:(o + 1) * TL], gRep[:])

            # ---------------- m slab (4 groups wide, boundary-zeroed) ----------
            mBig = const.tile([128, W4], F32, tag="mBig")
            nc.gpsimd.partition_broadcast(mBig[:], mrowT[:])
            nc.vector.tensor_scalar_mul(mBig[:], mBig[:], dcolT[:])

            # ---------------- front: w build + local scans + AG per slab --------
            # lf/init col layout per slab: col 2*jl (+1 for imag), group j = 8*s + jl
            lfAlls = []
            for v in range(8):
                s, h = v // 2, v % 2
                hsl = slice(h * W4, (h + 1) * W4)
                csl = slice(v * W4, (v + 1) * W4)
                cosS = slab.tile([128, W4], BF16, tag="tb", bufs=3, name=f"cosSw{v}")
                nc.sync.dma_start(out=cosS[:], in_=cosb[:, csl])
                sinS = slab.tile([128, W4], BF16, tag="tb", bufs=3, name=f"sinSw{v}")
                nc.sync.dma_start(out=sinS[:], in_=sinb[:, csl])
                wre = slab.tile([128, W4], BF16, tag="ws", bufs=3, name="wre")
                wim = slab.tile([128, W4], BF16, tag="ws", bufs=3, name="wim")
                nc.vector.tensor_tensor(out=wre[:], in0=gRep4p[:], in1=cosS[:], op=MULT)
                nc.vector.tensor_tensor(out=wim[:], in0=gRep4p[:], in1=sinS[:], op=MULT)
                nc.vector.tensor_tensor_scan(
                    out=uRe[s][:, hsl], data0=mBig[:], data1=wre[:],
                    initial=0.0, op0=MULT, op1=ADD)
                nc.vector.tensor_tensor_scan(
                    out=uIm[s][:, hsl], data0=mBig[:], data1=wim[:],
                    initial=0.0, op0=MULT, op1=ADD)
                if v % 2 == 0:
                    continue
                # slab s scans complete: local finals -> AllGather (async)
                lfS = const.tile([128, 16], F32, tag=f"lfS{s}", name=f"lfS{s}")
                nc.scalar.copy(out=lfS[:, 0:16:2], in_=uRe[s][:, TL - 1:: TL])
                nc.scalar.copy(out=lfS[:, 1:16:2], in_=uIm[s][:, TL - 1:: TL])
                nc.sync.dma_start(out=lfout[:, 16 * s:16 * (s + 1)], in_=lfS[:])
                lf_in_t = dramp.tile([128, 16], F32, tag=f"lf_in{s}", name=f"lf_in{s}")
                lf_all_t = dramp.tile([NCORE * 128, 16], F32, tag=f"lf_all{s}",
                                      name=f"lf_all{s}")
                nc.sync.dma_start(out=lf_in_t[:], in_=lfS[:])
                nc.gpsimd.collective_compute(
                    "AllGather", mybir.AluOpType.bypass,
                    replica_groups=[list(range(NCORE))],
                    ins=[lf_in_t.opt()], outs=[lf_all_t.opt()])
                lfAll = const.tile([128, NCORE * 16], F32, tag=f"lfAll{s}",
                                   name=f"lfAll{s}")
                nc.sync.dma_start(
                    out=lfAll[:].rearrange("p (r c) -> p r c", r=NCORE),
                    in_=lf_all_t[:].rearrange("(r p) c -> p r c", p=128))
                lfAlls.append(lfAll)

            # ---------------- back: chain -> correction per slab ----------------
            curs = []
            for s in range(4):
                lfAll = lfAlls[s]
                cur = init0T[:, 16 * s:16 * (s + 1)]
                for bb in range(NCORE):
                    tmp = smallp.tile([128, 16], F32, tag="chA", name=f"chA{s}_{bb}")
                    nc.vector.tensor_scalar_mul(tmp[:], cur, mf2T[:, bb:bb + 1])
                    nxt = smallp.tile([128, 16], F32, tag="chB", name=f"chB{s}_{bb}")
                    nc.vector.scalar_tensor_tensor(
                        out=nxt[:], in0=lfAll[:, bb * 16:(bb + 1) * 16],
                        scalar=mlfT[:, bb:bb + 1], in1=tmp[:], op0=MULT, op1=ADD)
                    cur = nxt[:]
                curF = smallp.tile([128, 16], F32, tag="curF", name=f"curF{s}")
                nc.vector.tensor_copy(curF[:], cur)
                curs.append(curF[:])
                # correction: tmp = dect * init (ACT, per-partition scale), u += tmp
                for jl in range(8):
                    jsl = slice(jl * TL, (jl + 1) * TL)
                    cre = slab.tile([128, TL], BF16, tag="ctmp", bufs=2, name=f"cre{s}_{jl}")
                    nc.scalar.activation(cre[:], dectT[:], AF.Copy,
                                         scale=curF[:, 2 * jl:2 * jl + 1])
                    nc.vector.tensor_tensor(out=uRe[s][:, jsl], in0=uRe[s][:, jsl],
                                            in1=cre[:], op=ADD)
                    cim = slab.tile([128, TL], BF16, tag="ctmp", bufs=2, name=f"cim{s}_{jl}")
                    nc.scalar.activation(cim[:], dectT[:], AF.Copy,
                                         scale=curF[:, 2 * jl + 1:2 * jl + 2])
                    nc.vector.tensor_tensor(out=uIm[s][:, jsl], in0=uIm[s][:, jsl],
                                            in1=cim[:], op=ADD)

            # ---------------- rotate-products + mixing matmul ----------------
            # zm = (cos.u)@A + (sn.u)@B with A/B chunk pairs per group
            zmP = [pz.tile([128, OUT], F32, tag=f"zm{t}", name=f"zmP{t}") for t in range(4)]
            for v in range(8):
                s, h = v // 2, v % 2
                hsl = slice(h * W4, (h + 1) * W4)
                csl = slice(v * W4, (v + 1) * W4)
                cosS = slab.tile([128, W4], BF16, tag="tb", bufs=3, name=f"cosSr{v}")
                nc.sync.dma_start(out=cosS[:], in_=cosb[:, csl])
                sinS = slab.tile([128, W4], BF16, tag="tb", bufs=3, name=f"sinSr{v}")
                nc.sync.dma_start(out=sinS[:], in_=sinb[:, csl])
                prods = []
                for name, uu, tabS in (("qcre", uRe[s], cosS), ("qcim", uIm[s], cosS),
                                       ("qsre", uRe[s], sinS), ("qsim", uIm[s], sinS)):
                    q = slab.tile([128, W4], BF16, tag="qs", bufs=5, name=f"{name}{v}")
                    nc.vector.tensor_tensor(out=q[:], in0=tabS[:],
                                            in1=uu[:, hsl], op=MULT)
                    prods.append(q)
                wmt = wmixp.tile([128, 16 * OUT], BF16, tag="wmt", name=f"wmt{v}")
                nc.sync.dma_start(out=wmt[:], in_=wmix[v])
                for jj in range(4):
                    j = 4 * v + jj
                    # block order: (qc_re, qc_im, qs_re, qs_im) per group
                    for qi, q in enumerate(prods):
                        idx = 4 * jj + qi
                        wsl = slice(idx * OUT, (idx + 1) * OUT)
                        first = (v == 0 and jj == 0 and qi == 0)
                        for tau in range(4):
                            nc.tensor.matmul(
                                zmP[tau][:],
                                q[:, jj * TL + tau * 128: jj * TL + (tau + 1) * 128],
                                wmt[:, wsl],
                                start=first, stop=(v == 7 and jj == 3 and qi == 3))
            # ---------------- gates / layernorm tail ----------------
            for tau in range(4):
                tsl = slice(tau * 128, (tau + 1) * 128)
                goP = pg.tile([128, OUT], F32, tag="goP")
                skP = pg.tile([128, OUT], F32, tag="skP")
                xg = []
                for kc in range(4):
                    xgt = slab.tile([128, 128], BF16, tag="xg", bufs=8, name=f"xg{tau}_{kc}")
                    nc.sync.dma_start(out=xgt[:], in_=xT[kc * 128:(kc + 1) * 128, tsl])
                    xg.append(xgt)
                    nc.tensor.matmul(goP[:], xgt[:], wgoT[kc][:],
                                     start=(kc == 0), stop=(kc == 3))
                for kc in range(4):
                    nc.tensor.matmul(skP[:], xg[kc][:], wskT[kc][:],
                                     start=(kc == 0), stop=(kc == 3))

                go2 = gp.tile([128, OUT], F32, tag="workf", name="go2")
                nc.vector.tensor_tensor(out=go2[:], in0=goP[:], in1=bgoRep[:], op=ADD)
                gate = gp.tile([128, OUT], F32, tag="gate", bufs=1)
                nc.scalar.activation(gate[:], go2[:], AF.Sigmoid)
                skipS = gp.tile([128, OUT], F32, tag="skipS", bufs=1)
                nc.vector.tensor_tensor(out=skipS[:], in0=skP[:], in1=bskRep[:], op=ADD)
                z2 = gp.tile([128, OUT], F32, tag="workf", name="z2")
                nc.vector.tensor_tensor(out=z2[:], in0=zmP[tau][:], in1=bmixRep[:], op=ADD)
                v_ = gp.tile([128, OUT], F32, tag="workf", name="v_")
                nc.vector.tensor_tensor(out=v_[:], in0=z2[:], in1=gate[:], op=MULT)
                musum = smallp.tile([128, 1], F32, tag="musum")
                nc.vector.tensor_reduce(out=musum[:], in_=v_[:], axis=mybir.AxisListType.X, op=ADD)
                negmu = smallp.tile([128, 1], F32, tag="negmu")
                nc.vector.tensor_scalar_mul(negmu[:], musum[:], -1.0 / OUT)
                cen = gp.tile([128, OUT], F32, tag="workf", name="cen")
                nc.scalar.activation(cen[:], v_[:], AF.Identity, bias=negmu[:], scale=1.0)
                sqj = gp.tile([128, OUT], BF16, tag="sqj", bufs=1)
                varsum = smallp.tile([128, 1], F32, tag="varsum")
                nc.scalar.activation(sqj[:], cen[:], AF.Square, accum_out=varsum[:])
                varm = smallp.tile([128, 1], F32, tag="varm")
                nc.vector.tensor_scalar(out=varm[:], in0=varsum[:], scalar1=1.0 / OUT,
                                        scalar2=LN_EPS, op0=MULT, op1=ADD)
                stdc = smallp.tile([128, 1], F32, tag="stdc")
                nc.scalar.activation(stdc[:], varm[:], AF.Sqrt)
                rstd = smallp.tile([128, 1], F32, tag="rstd")
                nc.vector.reciprocal(rstd[:], stdc[:])
                ln = gp.tile([128, OUT], F32, tag="workf", name="ln")
                nc.vector.tensor_scalar_mul(ln[:], cen[:], rstd[:])
                omg = gp.tile([128, OUT], F32, tag="workf", name="omg")
                nc.vector.tensor_scalar(out=omg[:], in0=gate[:], scalar1=-1.0,
                                        scalar2=1.0, op0=MULT, op1=ADD)
                t5 = gp.tile([128, OUT], F32, tag="workf", name="t5")
                nc.vector.tensor_tensor(out=t5[:], in0=skipS[:], in1=omg[:], op=MULT)
                outT = gp.tile([128, OUT], F32, tag="workf", name="outT")
                nc.vector.tensor_tensor(out=outT[:], in0=ln[:], in1=t5[:], op=ADD)
                nc.sync.dma_start(out=out_d[tsl, :], in_=outT[:])

    nc.finalize()
    return nc


_NC_CACHE = {}


def _get_nc():
    if "nc" not in _NC_CACHE:
        _NC_CACHE["nc"] = _build_program()
    return _NC_CACHE["nc"]


def _host_prep(inputs):
    f8 = np.float64
    x = np.asarray(inputs["x"], f8)
    resets = np.asarray(inputs["resets"]).astype(bool)
    a = np.abs(np.asarray(inputs["a"], f8))
    b = np.asarray(inputs["b"], f8)
    s0 = (np.asarray(inputs["state_re"], f8)[0]
          + 1j * np.asarray(inputs["state_im"], f8)[0])      # [TR, CX]

    W_pre = np.asarray(inputs["W_pre"], f8)
    W_gi = np.asarray(inputs["W_gi"], f8)
    wpre2 = np.concatenate([W_pre, W_pre], 1).astype(NP_BF16)
    wgi2 = np.concatenate([W_gi, W_gi], 1).astype(NP_BF16)
    bprecol = np.tile(np.asarray(inputs["b_pre"], f8), 2)[:, None].astype(np.float32)
    bgicol = np.tile(np.asarray(inputs["b_gi"], f8), 2)[:, None].astype(np.float32)
    wgo = np.asarray(inputs["W_go"], f8).astype(NP_BF16)
    wskip = np.asarray(inputs["W_skip"], f8).astype(NP_BF16)
    brow = np.stack([np.asarray(inputs["b_go"], f8),
                     np.asarray(inputs["b_skip"], f8),
                     np.asarray(inputs["b_mix"], f8)]).astype(np.float32)

    # channel permutation tables
    c_of = 2 * np.arange(G)[None, :] + HI_OF_P[:, None]       # [128, G]
    bmat = b[c_of]                                            # [128, G]
    Wm = np.asarray(inputs["W_mix"], f8)
    # mega-blocks: block v holds, for j = 4v..4v+3, the four OUT-wide chunks
    # pairing (cos.u_re, cos.u_im, sn.u_re, sn.u_im) = (Wr, Wi, -Wi, Wr)
    wmix = np.empty((8, 128, 16 * OUT), NP_BF16)
    for j in range(G):
        c = c_of[:, j]
        Wr = Wm[TR_OF_P * 128 + c]
        Wi = Wm[TR_OF_P * 128 + 64 + c]
        v, jj = j // 4, j % 4
        wmix[v, :, (4 * jj + 0) * OUT:(4 * jj + 1) * OUT] = Wr.astype(NP_BF16)
        wmix[v, :, (4 * jj + 1) * OUT:(4 * jj + 2) * OUT] = Wi.astype(NP_BF16)
        wmix[v, :, (4 * jj + 2) * OUT:(4 * jj + 3) * OUT] = (-Wi).astype(NP_BF16)
        wmix[v, :, (4 * jj + 3) * OUT:(4 * jj + 4) * OUT] = Wr.astype(NP_BF16)

    decay = np.exp(-a)                                        # [TR]
    dcol = decay[TR_OF_P][:, None].astype(np.float32)
    anyr = np.array([resets[k * TL:(k + 1) * TL].any() for k in range(NCORE)])
    Mf_part = np.exp(-TL * a[TR_OF_P])                        # [128]

    init0c = np.exp(1j * bmat) * s0[TR_OF_P[:, None], c_of]   # [128, G] complex
    # lf/init column layout: col 16*s + 2*jj (+1) for group j = 8*s + jj
    col_of_j = np.array([16 * (j // 8) + 2 * (j % 8) for j in range(G)], np.int64)
    init0 = np.empty((128, 2 * G), np.float32)
    init0[:, col_of_j] = init0c.real
    init0[:, col_of_j + 1] = init0c.imag

    in_maps = []
    for k in range(NCORE):
        tg = np.arange(k * TL, (k + 1) * TL, dtype=f8)        # global t
        ph = bmat[:, :, None] * tg[None, None, :]             # [128, G, TL]
        cosb = np.cos(ph).reshape(128, G * TL).astype(NP_BF16)
        sinb = (-np.sin(ph)).reshape(128, G * TL).astype(NP_BF16)
        notr = (~resets[k * TL:(k + 1) * TL]).astype(f8)      # [TL]
        cumnr = np.cumprod(notr)
        dect = (np.exp(-a[TR_OF_P][:, None] * (np.arange(TL)[None, :] + 1))
                * cumnr[None, :]).astype(NP_BF16)
        mrow = np.tile(notr, 4).astype(np.float32)
        mrow[0::TL] = 0.0                                     # group boundaries
        mf2 = np.empty((128, NCORE), np.float32)
        mlf = np.empty((128, NCORE), np.float32)
        for bb in range(NCORE):
            if bb < k:
                mf2[:, bb] = Mf_part * (0.0 if anyr[bb] else 1.0)
                mlf[:, bb] = 1.0
            else:
                mf2[:, bb] = 1.0
                mlf[:, bb] = 0.0
        in_maps.append(dict(
            xT=np.ascontiguousarray(x[k * TL:(k + 1) * TL].T).astype(NP_BF16),
            wpre2=wpre2, wgi2=wgi2, bprecol=bprecol, bgicol=bgicol,
            wgo=wgo, wskip=wskip, brow=brow, wmix=wmix,
            cosb=cosb, sinb=sinb, dect=dect,
            mrow=mrow[None, :],
            dcol=dcol, init0=init0, mf2=mf2, mlf=mlf,
        ))
    aux = dict(bmat=bmat, Mf_part=Mf_part, anyr=anyr, init0c=init0c, c_of=c_of,
               col_of_j=col_of_j)
    return in_maps, aux


def _assemble(results, aux):
    out = np.concatenate([results[k]["out"] for k in range(NCORE)], 0).astype(np.float32)

    # final state: chain the device-produced local finals on the host
    col = aux["col_of_j"]
    init = aux["init0c"].astype(np.complex128)                # [128, G]
    for k in range(NCORE):
        lf = results[k]["lfout"]
        lfc = lf[:, col] + 1j * lf[:, col + 1]
        init = lfc + (0.0 if aux["anyr"][k] else 1.0) * aux["Mf_part"][:, None] * init
    sfin = init * np.exp(1j * aux["bmat"] * (T - 1))
    fin = np.zeros((TR, CX), np.complex64)
    fin[TR_OF_P[:, None], aux["c_of"]] = sfin.astype(np.complex64)
    return fin[None], out


def kernel(**inputs):
    nc = _get_nc()
    in_maps, aux = _host_prep(inputs)
    res = run_bass_kernel_spmd(nc, in_maps, list(range(NCORE)))
    return _assemble(res.results, aux)


# revision 26
# speedup vs baseline: 1.2787x; 1.0222x over previous
# Trainium2 Bass kernel for the FFM (fast-forgetful-memory) layer.
#
# Math: the reference does a complex segmented scan  s[t] = z[t] + !r[t] * gamma * s[t-1]
# over channels (trace=64) x (ctx=64), gamma = exp(-|a_tr| + i*b_c), followed by a
# big mixing matmul.  We substitute u[t] = exp(-i*b_c*t) * s[t], which turns the
# complex recurrence into two independent REAL first-order scans
#     u[t] = exp(-|a|) * !r[t] * u[t-1] + exp(-i*b*t) * g[t]
# (numerically stable for the whole sequence since |exp(-i*b*t)| = 1).  These map
# directly onto the DVE tensor_tensor_scan instruction.
#
# The output rotation s = exp(i*b*t) * u is folded into the mixing matmul: with
# sn := -sin(b*t),
#   zm = sum_ch s_re*Wr + s_im*Wi
#      = sum_ch (cos.u_re)Wr + (cos.u_im)Wi + (sn.u_re)(-Wi) + (sn.u_im)Wr
# so only TWO elementwise products per u component are needed (cos.u and sn.u),
# each contracted against a normal (A) and a swapped/negated (B) copy of W_mix.
#
# Sharding: time is split across the 8 cores (512 steps each).  Each core runs
# zero-initialized local scans, per-core carry summaries are exchanged with two
# batched AllGathers (latency hidden behind the remaining scans), a short
# on-device chain reconstructs each core's true initial state, and a rank-1
# correction (on GPSIMD) fixes the local scans.
import numpy as np

import concourse.bass as bass
import concourse.bacc as bacc
import concourse.mybir as mybir
import concourse.tile as tile
from concourse.bass_utils import run_bass_kernel_spmd

T, D, TR, CX, OUT = 4096, 512, 64, 64, 512
NCORE, TL, G = 8, 512, 32
LN_EPS = 1e-6
W4 = 4 * TL   # 4-group slab width (2048)

F32 = mybir.dt.float32
BF16 = mybir.dt.bfloat16
NP_BF16 = mybir.dt.np(BF16)
MULT = mybir.AluOpType.mult
ADD = mybir.AluOpType.add
SUB = mybir.AluOpType.subtract
AF = mybir.ActivationFunctionType

_P = np.arange(128)
TR_OF_P = _P % 64            # trace index per partition
HI_OF_P = _P // 64           # 0/1 selecting c = 2j + HI


def _build_program():
    nc = bacc.Bacc("TRN2", target_bir_lowering=False, debug=False, num_devices=NCORE)

    def inp(name, shape, dtype):
        return nc.dram_tensor(name, shape, dtype, kind="ExternalInput").ap()

    xT = inp("xT", [D, TL], BF16)
    wpre2 = inp("wpre2", [D, 128], BF16)
    wgi2 = inp("wgi2", [D, 128], BF16)
    bprecol = inp("bprecol", [128, 1], F32)
    bgicol = inp("bgicol", [128, 1], F32)
    wgo = inp("wgo", [D, OUT], BF16)
    wskip = inp("wskip", [D, OUT], BF16)
    brow = inp("brow", [3, OUT], F32)
    wmix = inp("wmix", [16, 128, 8 * OUT], BF16)  # consumption-ordered blocks
    cosb = inp("cosb", [128, G * TL], BF16)
    sinb = inp("sinb", [128, G * TL], BF16)     # = -sin(b*t)
    dect = inp("dect", [128, TL], BF16)
    mrow = inp("mrow", [1, W4], F32)            # notreset, zeroed at 512-boundaries
    dcol = inp("dcol", [128, 1], F32)
    init0 = inp("init0", [128, 2 * G], F32)
    mf2 = inp("mf2", [128, NCORE], F32)
    mlf = inp("mlf", [128, NCORE], F32)

    out_d = nc.dram_tensor("out", [TL, OUT], F32, kind="ExternalOutput").ap()
    lfout = nc.dram_tensor("lfout", [128, 2 * G], F32, kind="ExternalOutput").ap()

    with tile.TileContext(nc) as tc:
        with (
            tc.tile_pool(name="big", bufs=1) as big,
            tc.tile_pool(name="const", bufs=1) as const,
            tc.tile_pool(name="slab", bufs=1) as slab,  # per-tag bufs overrides
            tc.tile_pool(name="wmixp", bufs=2) as wmixp,
            tc.tile_pool(name="gp", bufs=4) as gp,
            tc.tile_pool(name="smallp", bufs=2) as smallp,
            tc.tile_pool(name="pz", bufs=1, space="PSUM") as pz,
            tc.tile_pool(name="pg", bufs=1, space="PSUM") as pg,
            tc.tile_pool(name="dramp", bufs=1, space="DRAM") as dramp,
        ):
            # ---------------- persistent loads ----------------
            uRe = [big.tile([128, 8 * TL], BF16, tag=f"uRe{s}", name=f"uRe{s}") for s in range(4)]
            uIm = [big.tile([128, 8 * TL], BF16, tag=f"uIm{s}", name=f"uIm{s}") for s in range(4)]

            wgoT, wskT = [], []
            for kc in range(4):
                t = const.tile([128, OUT], BF16, tag=f"wgo{kc}", name=f"wgoT{kc}")
                nc.sync.dma_start(out=t[:], in_=wgo[kc * 128:(kc + 1) * 128, :])
                wgoT.append(t)
                t = const.tile([128, OUT], BF16, tag=f"wsk{kc}", name=f"wskT{kc}")
                nc.sync.dma_start(out=t[:], in_=wskip[kc * 128:(kc + 1) * 128, :])
                wskT.append(t)
            dectT = const.tile([128, TL], BF16, tag="dect")
            nc.sync.dma_start(out=dectT[:], in_=dect[:])
            mrowT = const.tile([1, W4], F32, tag="mrow")
            nc.sync.dma_start(out=mrowT[:], in_=mrow[:])
            dcolT = const.tile([128, 1], F32, tag="dcol")
            nc.sync.dma_start(out=dcolT[:], in_=dcol[:])
            bpreT = const.tile([128, 1], F32, tag="bpre")
            nc.sync.dma_start(out=bpreT[:], in_=bprecol[:])
            bgiT = const.tile([128, 1], F32, tag="bgi")
            nc.sync.dma_start(out=bgiT[:], in_=bgicol[:])
            bgoR = const.tile([1, OUT], F32, tag="bgoR")
            nc.sync.dma_start(out=bgoR[:], in_=brow[0:1, :])
            bskR = const.tile([1, OUT], F32, tag="bskR")
            nc.sync.dma_start(out=bskR[:], in_=brow[1:2, :])
            bmixR = const.tile([1, OUT], F32, tag="bmixR")
            nc.sync.dma_start(out=bmixR[:], in_=brow[2:3, :])
            bgoB = const.tile([1, OUT], BF16, tag="bgoB")
            nc.vector.tensor_copy(bgoB[:], bgoR[:])
            bskB = const.tile([1, OUT], BF16, tag="bskB")
            nc.vector.tensor_copy(bskB[:], bskR[:])
            bmixB = const.tile([1, OUT], BF16, tag="bmixB")
            nc.vector.tensor_copy(bmixB[:], bmixR[:])
            bgoRep = const.tile([128, OUT], BF16, tag="bgoRep")
            nc.gpsimd.partition_broadcast(bgoRep[:], bgoB[:])
            bskRep = const.tile([128, OUT], BF16, tag="bskRep")
            nc.gpsimd.partition_broadcast(bskRep[:], bskB[:])
            bmixRep = const.tile([128, OUT], BF16, tag="bmixRep")
            nc.gpsimd.partition_broadcast(bmixRep[:], bmixB[:])
            init0T = const.tile([128, 2 * G], F32, tag="init0")
            nc.sync.dma_start(out=init0T[:], in_=init0[:])
            mf2T = const.tile([128, NCORE], F32, tag="mf2")
            nc.sync.dma_start(out=mf2T[:], in_=mf2[:])
            mlfT = const.tile([128, NCORE], F32, tag="mlf")
            nc.sync.dma_start(out=mlfT[:], in_=mlf[:])

            # ---------------- g = pre * sigmoid(gi), doubled over partitions ----
            ppre = pg.tile([128, TL], F32, tag="ppre", bufs=2)
            pgi = pg.tile([128, TL], F32, tag="pgi", bufs=2)
            for kc in range(4):
                xc = slab.tile([128, TL], BF16, tag="xc", bufs=2, name=f"xc{kc}")
                nc.sync.dma_start(out=xc[:], in_=xT[kc * 128:(kc + 1) * 128, :])
                wt = const.tile([128, 128], BF16, tag=f"wpre{kc}", name=f"wt_{kc}")
                nc.sync.dma_start(out=wt[:], in_=wpre2[kc * 128:(kc + 1) * 128, :])
                nc.tensor.matmul(ppre[:], wt[:], xc[:], start=(kc == 0), stop=(kc == 3))
                wt2 = const.tile([128, 128], BF16, tag=f"wgi{kc}", name=f"wt2_{kc}")
                nc.sync.dma_start(out=wt2[:], in_=wgi2[kc * 128:(kc + 1) * 128, :])
                nc.tensor.matmul(pgi[:], wt2[:], xc[:], start=(kc == 0), stop=(kc == 3))
            preB = gp.tile([128, TL], F32, tag="workf", name="preB")
            nc.scalar.activation(preB[:], ppre[:], AF.Identity, bias=bpreT[:], scale=1.0)
            giS = gp.tile([128, TL], F32, tag="workf", name="giS")
            nc.scalar.activation(giS[:], pgi[:], AF.Sigmoid, bias=bgiT[:], scale=1.0)
            gRep = const.tile([128, TL], BF16, tag="gRep")
            nc.vector.tensor_tensor(out=gRep[:], in0=preB[:], in1=giS[:], op=MULT)
            gRep4p = const.tile([128, W4], BF16, tag="gRep4p")
            for o in range(4):
                nc.vector.tensor_copy(gRep4p[:, o * TL:(o + 1) * TL], gRep[:])

            # ---------------- m slab (4 groups wide, boundary-zeroed) ----------
            mBig = const.tile([128, W4], F32, tag="mBig")
            nc.gpsimd.partition_broadcast(mBig[:], mrowT[:])
            nc.vector.tensor_scalar_mul(mBig[:], mBig[:], dcolT[:])

            # ---------------- front: w build + local scans + AG per slab --------
            # lf/init col layout per slab: col 2*jl (+1 for imag), group j = 8*s + jl
            lfAlls = []
            for v in range(8):
                s, h = v // 2, v % 2
                hsl = slice(h * W4, (h + 1) * W4)
                csl = slice(v * W4, (v + 1) * W4)
                cosS = slab.tile([128, W4], BF16, tag="tb", bufs=2, name=f"cosSw{v}")
                nc.sync.dma_start(out=cosS[:], in_=cosb[:, csl])
                sinS = slab.tile([128, W4], BF16, tag="tb", bufs=2, name=f"sinSw{v}")
                nc.sync.dma_start(out=sinS[:], in_=sinb[:, csl])
                wre = slab.tile([128, W4], BF16, tag="ws", bufs=2, name="wre")
                wim = slab.tile([128, W4], BF16, tag="ws", bufs=2, name="wim")
                nc.vector.tensor_tensor(out=wre[:], in0=gRep4p[:], in1=cosS[:], op=MULT)
                nc.vector.tensor_tensor(out=wim[:], in0=gRep4p[:], in1=sinS[:], op=MULT)
                nc.vector.tensor_tensor_scan(
                    out=uRe[s][:, hsl], data0=mBig[:], data1=wre[:],
                    initial=0.0, op0=MULT, op1=ADD)
                nc.vector.tensor_tensor_scan(
                    out=uIm[s][:, hsl], data0=mBig[:], data1=wim[:],
                    initial=0.0, op0=MULT, op1=ADD)
                if v % 2 == 0:
                    continue
                # slab s scans complete: local finals -> AllGather (async)
                lfS = const.tile([128, 16], F32, tag=f"lfS{s}", name=f"lfS{s}")
                nc.scalar.copy(out=lfS[:, 0:16:2], in_=uRe[s][:, TL - 1:: TL])
                nc.scalar.copy(out=lfS[:, 1:16:2], in_=uIm[s][:, TL - 1:: TL])
                nc.sync.dma_start(out=lfout[:, 16 * s:16 * (s + 1)], in_=lfS[:])
                lf_in_t = dramp.tile([128, 16], F32, tag=f"lf_in{s}", name=f"lf_in{s}")
                lf_all_t = dramp.tile([NCORE * 128, 16], F32, tag=f"lf_all{s}",
                                      name=f"lf_all{s}")
                nc.sync.dma_start(out=lf_in_t[:], in_=lfS[:])
                nc.gpsimd.collective_compute(
                    "AllGather", mybir.AluOpType.bypass,
                    replica_groups=[list(range(NCORE))],
                    ins=[lf_in_t.opt()], outs=[lf_all_t.opt()])
                lfAll = const.tile([128, NCORE * 16], F32, tag=f"lfAll{s}",
                                   name=f"lfAll{s}")
                nc.sync.dma_start(
                    out=lfAll[:].rearrange("p (r c) -> p r c", r=NCORE),
                    in_=lf_all_t[:].rearrange("(r p) c -> p r c", p=128))
                lfAlls.append(lfAll)

            # ---------------- back: chain -> correction per slab ----------------
            curs = []
            for s in range(4):
                lfAll = lfAlls[s]
                cur = init0T[:, 16 * s:16 * (s + 1)]
                for bb in range(NCORE):
                    tmp = smallp.tile([128, 16], F32, tag="chA", name=f"chA{s}_{bb}")
                    nc.vector.tensor_scalar_mul(tmp[:], cur, mf2T[:, bb:bb + 1])
                    nxt = smallp.tile([128, 16], F32, tag="chB", name=f"chB{s}_{bb}")
                    nc.vector.scalar_tensor_tensor(
                        out=nxt[:], in0=lfAll[:, bb * 16:(bb + 1) * 16],
                        scalar=mlfT[:, bb:bb + 1], in1=tmp[:], op0=MULT, op1=ADD)
                    cur = nxt[:]
                curF = smallp.tile([128, 16], F32, tag="curF", name=f"curF{s}")
                nc.vector.tensor_copy(curF[:], cur)
                curs.append(curF[:])
                # correction: tmp = dect * init (ACT, per-partition scale), u += tmp
                for jl in range(8):
                    jsl = slice(jl * TL, (jl + 1) * TL)
                    cre = slab.tile([128, TL], BF16, tag="ctmp", bufs=2, name=f"cre{s}_{jl}")
                    nc.scalar.activation(cre[:], dectT[:], AF.Copy,
                                         scale=curF[:, 2 * jl:2 * jl + 1])
                    nc.vector.tensor_tensor(out=uRe[s][:, jsl], in0=uRe[s][:, jsl],
                                            in1=cre[:], op=ADD)
                    cim = slab.tile([128, TL], BF16, tag="ctmp", bufs=2, name=f"cim{s}_{jl}")
                    nc.scalar.activation(cim[:], dectT[:], AF.Copy,
                                         scale=curF[:, 2 * jl + 1:2 * jl + 2])
                    nc.vector.tensor_tensor(out=uIm[s][:, jsl], in0=uIm[s][:, jsl],
                                            in1=cim[:], op=ADD)

            # ---------------- rotate-products + mixing matmul ----------------
            # zm = (cos.u)@A + (sn.u)@B with A/B chunk pairs per group
            zmP = [pz.tile([128, OUT], F32, tag=f"zm{t}", name=f"zmP{t}") for t in range(4)]
            for v in range(8):
                s, h = v // 2, v % 2
                hsl = slice(h * W4, (h + 1) * W4)
                csl = slice(v * W4, (v + 1) * W4)
                cosS = slab.tile([128, W4], BF16, tag="tb", bufs=2, name=f"cosSr{v}")
                nc.sync.dma_start(out=cosS[:], in_=cosb[:, csl])
                sinS = slab.tile([128, W4], BF16, tag="tb", bufs=2, name=f"sinSr{v}")
                nc.sync.dma_start(out=sinS[:], in_=sinb[:, csl])
                prods = []
                for name, uu, tabS in (("qcre", uRe[s], cosS), ("qcim", uIm[s], cosS),
                                       ("qsre", uRe[s], sinS), ("qsim", uIm[s], sinS)):
                    q = slab.tile([128, W4], BF16, tag="qs", bufs=4, name=f"{name}{v}")
                    nc.vector.tensor_tensor(out=q[:], in0=tabS[:],
                                            in1=uu[:, hsl], op=MULT)
                    prods.append(q)
                for half in range(2):
                    wmt = wmixp.tile([128, 8 * OUT], BF16, tag="wmt", name=f"wmt{v}_{half}")
                    nc.sync.dma_start(out=wmt[:], in_=wmix[2 * v + half])
                    for jj in (2 * half, 2 * half + 1):
                        j = 4 * v + jj
                        # block order: (qc_re, qc_im, qs_re, qs_im) per group
                        for qi, q in enumerate(prods):
                            idx = 4 * (jj % 2) + qi
                            wsl = slice(idx * OUT, (idx + 1) * OUT)
                            first = (v == 0 and jj == 0 and qi == 0)
                            for tau in range(4):
                                nc.tensor.matmul(
                                    zmP[tau][:],
                                    q[:, jj * TL + tau * 128: jj * TL + (tau + 1) * 128],
                                    wmt[:, wsl],
                                    start=first, stop=(v == 7 and jj == 3 and qi == 3))
            # ---------------- gate/skip (zm-independent, PE warm-up fill) -------
            gates, t5s = [], []
            for tau in range(4):
                tsl = slice(tau * 128, (tau + 1) * 128)
                goP = pg.tile([128, OUT], F32, tag="ppre", bufs=2, name=f"goP{tau}")
                skP = pg.tile([128, OUT], F32, tag="pgi", bufs=2, name=f"skP{tau}")
                xg = []
                for kc in range(4):
                    xgt = slab.tile([128, 128], BF16, tag="xg", bufs=8, name=f"xg{tau}_{kc}")
                    nc.sync.dma_start(out=xgt[:], in_=xT[kc * 128:(kc + 1) * 128, tsl])
                    xg.append(xgt)
                    nc.tensor.matmul(goP[:], xgt[:], wgoT[kc][:],
                                     start=(kc == 0), stop=(kc == 3))
                for kc in range(4):
                    nc.tensor.matmul(skP[:], xg[kc][:], wskT[kc][:],
                                     start=(kc == 0), stop=(kc == 3))
                go2 = gp.tile([128, OUT], F32, tag="workf", name="go2")
                nc.vector.tensor_tensor(out=go2[:], in0=goP[:], in1=bgoRep[:], op=ADD)
                gate = gp.tile([128, OUT], F32, tag="gate", bufs=4, name=f"gate{tau}")
                nc.scalar.activation(gate[:], go2[:], AF.Sigmoid)
                skipS = gp.tile([128, OUT], F32, tag="skipS", bufs=1, name=f"skipS{tau}")
                nc.vector.tensor_tensor(out=skipS[:], in0=skP[:], in1=bskRep[:], op=ADD)
                omg = gp.tile([128, OUT], F32, tag="omg", bufs=1, name=f"omg{tau}")
                nc.vector.tensor_scalar(out=omg[:], in0=gate[:], scalar1=-1.0,
                                        scalar2=1.0, op0=MULT, op1=ADD)
                t5 = gp.tile([128, OUT], F32, tag="t5", bufs=4, name=f"t5{tau}")
                nc.vector.tensor_tensor(out=t5[:], in0=skipS[:], in1=omg[:], op=MULT)
                gates.append(gate)
                t5s.append(t5)

            # ---------------- layernorm tail ----------------
            for tau in range(4):
                tsl = slice(tau * 128, (tau + 1) * 128)
                gate = gates[tau]
                t5 = t5s[tau]
                z2 = gp.tile([128, OUT], F32, tag="workf", name="z2")
                nc.vector.tensor_tensor(out=z2[:], in0=zmP[tau][:], in1=bmixRep[:], op=ADD)
                v_ = gp.tile([128, OUT], F32, tag="workf", name="v_")
                nc.vector.tensor_tensor(out=v_[:], in0=z2[:], in1=gate[:], op=MULT)
                musum = smallp.tile([128, 1], F32, tag="musum")
                nc.vector.tensor_reduce(out=musum[:], in_=v_[:], axis=mybir.AxisListType.X, op=ADD)
                negmu = smallp.tile([128, 1], F32, tag="negmu")
                nc.vector.tensor_scalar_mul(negmu[:], musum[:], -1.0 / OUT)
                cen = gp.tile([128, OUT], F32, tag="workf", name="cen")
                nc.scalar.activation(cen[:], v_[:], AF.Identity, bias=negmu[:], scale=1.0)
                sqj = gp.tile([128, OUT], BF16, tag="sqj", bufs=1)
                varsum = smallp.tile([128, 1], F32, tag="varsum")
                nc.scalar.activation(sqj[:], cen[:], AF.Square, accum_out=varsum[:])
                varm = smallp.tile([128, 1], F32, tag="varm")
                nc.vector.tensor_scalar(out=varm[:], in0=varsum[:], scalar1=1.0 / OUT,
                                        scalar2=LN_EPS, op0=MULT, op1=ADD)
                stdc = smallp.tile([128, 1], F32, tag="stdc")
                nc.scalar.activation(stdc[:], varm[:], AF.Sqrt)
                rstd = smallp.tile([128, 1], F32, tag="rstd")
                nc.vector.reciprocal(rstd[:], stdc[:])
                ln = gp.tile([128, OUT], F32, tag="workf", name="ln")
                nc.vector.tensor_scalar_mul(ln[:], cen[:], rstd[:])
                outT = gp.tile([128, OUT], F32, tag="workf", name="outT")
                nc.vector.tensor_tensor(out=outT[:], in0=ln[:], in1=t5[:], op=ADD)
                nc.sync.dma_start(out=out_d[tsl, :], in_=outT[:])

    nc.finalize()
    return nc


_NC_CACHE = {}


def _get_nc():
    if "nc" not in _NC_CACHE:
        _NC_CACHE["nc"] = _build_program()
    return _NC_CACHE["nc"]


def _host_prep(inputs):
    f8 = np.float64
    x = np.asarray(inputs["x"], f8)
    resets = np.asarray(inputs["resets"]).astype(bool)
    a = np.abs(np.asarray(inputs["a"], f8))
    b = np.asarray(inputs["b"], f8)
    s0 = (np.asarray(inputs["state_re"], f8)[0]
          + 1j * np.asarray(inputs["state_im"], f8)[0])      # [TR, CX]

    W_pre = np.asarray(inputs["W_pre"], f8)
    W_gi = np.asarray(inputs["W_gi"], f8)
    wpre2 = np.concatenate([W_pre, W_pre], 1).astype(NP_BF16)
    wgi2 = np.concatenate([W_gi, W_gi], 1).astype(NP_BF16)
    bprecol = np.tile(np.asarray(inputs["b_pre"], f8), 2)[:, None].astype(np.float32)
    bgicol = np.tile(np.asarray(inputs["b_gi"], f8), 2)[:, None].astype(np.float32)
    wgo = np.asarray(inputs["W_go"], f8).astype(NP_BF16)
    wskip = np.asarray(inputs["W_skip"], f8).astype(NP_BF16)
    brow = np.stack([np.asarray(inputs["b_go"], f8),
                     np.asarray(inputs["b_skip"], f8),
                     np.asarray(inputs["b_mix"], f8)]).astype(np.float32)

    # channel permutation tables
    c_of = 2 * np.arange(G)[None, :] + HI_OF_P[:, None]       # [128, G]
    bmat = b[c_of]                                            # [128, G]
    Wm = np.asarray(inputs["W_mix"], f8)
    # mega-blocks: block v holds, for j = 4v..4v+3, the four OUT-wide chunks
    # pairing (cos.u_re, cos.u_im, sn.u_re, sn.u_im) = (Wr, Wi, -Wi, Wr)
    wmix = np.empty((16, 128, 8 * OUT), NP_BF16)
    for j in range(G):
        c = c_of[:, j]
        Wr = Wm[TR_OF_P * 128 + c]
        Wi = Wm[TR_OF_P * 128 + 64 + c]
        v, jj = j // 4, j % 4
        bi = 2 * v + jj // 2
        o = 4 * (jj % 2)
        wmix[bi, :, (o + 0) * OUT:(o + 1) * OUT] = Wr.astype(NP_BF16)
        wmix[bi, :, (o + 1) * OUT:(o + 2) * OUT] = Wi.astype(NP_BF16)
        wmix[bi, :, (o + 2) * OUT:(o + 3) * OUT] = (-Wi).astype(NP_BF16)
        wmix[bi, :, (o + 3) * OUT:(o + 4) * OUT] = Wr.astype(NP_BF16)

    decay = np.exp(-a)                                        # [TR]
    dcol = decay[TR_OF_P][:, None].astype(np.float32)
    anyr = np.array([resets[k * TL:(k + 1) * TL].any() for k in range(NCORE)])
    Mf_part = np.exp(-TL * a[TR_OF_P])                        # [128]

    init0c = np.exp(1j * bmat) * s0[TR_OF_P[:, None], c_of]   # [128, G] complex
    # lf/init column layout: col 16*s + 2*jj (+1) for group j = 8*s + jj
    col_of_j = np.array([16 * (j // 8) + 2 * (j % 8) for j in range(G)], np.int64)
    init0 = np.empty((128, 2 * G), np.float32)
    init0[:, col_of_j] = init0c.real
    init0[:, col_of_j + 1] = init0c.imag

    in_maps = []
    for k in range(NCORE):
        tg = np.arange(k * TL, (k + 1) * TL, dtype=f8)        # global t
        ph = bmat[:, :, None] * tg[None, None, :]             # [128, G, TL]
        cosb = np.cos(ph).reshape(128, G * TL).astype(NP_BF16)
        sinb = (-np.sin(ph)).reshape(128, G * TL).astype(NP_BF16)
        notr = (~resets[k * TL:(k + 1) * TL]).astype(f8)      # [TL]
        cumnr = np.cumprod(notr)
        dect = (np.exp(-a[TR_OF_P][:, None] * (np.arange(TL)[None, :] + 1))
                * cumnr[None, :]).astype(NP_BF16)
        mrow = np.tile(notr, 4).astype(np.float32)
        mrow[0::TL] = 0.0                                     # group boundaries
        mf2 = np.empty((128, NCORE), np.float32)
        mlf = np.empty((128, NCORE), np.float32)
        for bb in range(NCORE):
            if bb < k:
                mf2[:, bb] = Mf_part * (0.0 if anyr[bb] else 1.0)
                mlf[:, bb] = 1.0
            else:
                mf2[:, bb] = 1.0
                mlf[:, bb] = 0.0
        in_maps.append(dict(
            xT=np.ascontiguousarray(x[k * TL:(k + 1) * TL].T).astype(NP_BF16),
            wpre2=wpre2, wgi2=wgi2, bprecol=bprecol, bgicol=bgicol,
            wgo=wgo, wskip=wskip, brow=brow, wmix=wmix,
            cosb=cosb, sinb=sinb, dect=dect,
            mrow=mrow[None, :],
            dcol=dcol, init0=init0, mf2=mf2, mlf=mlf,
        ))
    aux = dict(bmat=bmat, Mf_part=Mf_part, anyr=anyr, init0c=init0c, c_of=c_of,
               col_of_j=col_of_j)
    return in_maps, aux


def _assemble(results, aux):
    out = np.concatenate([results[k]["out"] for k in range(NCORE)], 0).astype(np.float32)

    # final state: chain the device-produced local finals on the host
    col = aux["col_of_j"]
    init = aux["init0c"].astype(np.complex128)                # [128, G]
    for k in range(NCORE):
        lf = results[k]["lfout"]
        lfc = lf[:, col] + 1j * lf[:, col + 1]
        init = lfc + (0.0 if aux["anyr"][k] else 1.0) * aux["Mf_part"][:, None] * init
    sfin = init * np.exp(1j * aux["bmat"] * (T - 1))
    fin = np.zeros((TR, CX), np.complex64)
    fin[TR_OF_P[:, None], aux["c_of"]] = sfin.astype(np.complex64)
    return fin[None], out


def kernel(**inputs):
    nc = _get_nc()
    in_maps, aux = _host_prep(inputs)
    res = run_bass_kernel_spmd(nc, in_maps, list(range(NCORE)))
    return _assemble(res.results, aux)
